# revision 11
# baseline (speedup 1.0000x reference)
"""Symmetric-halved Euclidean distance matrix on 8 Trainium2 NeuronCores.

Decomposition: 16 column strips of 512. Core c owns strips 2c, 2c+1 and
computes, for each owned strip s, the blocks d(rows strip (s+d) mod 16,
cols strip s) for diagonal offsets d = 0..8. Every unordered strip pair
{u, v} is covered (offset (v-u) mod 16 <= 8 exactly once, except offset-8
pairs computed twice - harmless). The host mirrors each [512, 512] block to
its transposed position, so only ~59% of the matrix is computed on device.

The core's input is one local window xj = X^T columns for strips
2c..2c+9 (mod 16) [512, 5120]; all addressing inside the kernel uses local
strip indices 0..9, so the program is SPMD-uniform.
"""
import sys

sys.path.insert(0, "/opt/trn_rl_repo")

import numpy as np

N, D, NCORES = 8192, 512, 8
P = 128
KO = D // P          # 4 contraction blocks
NSTRIP = 16          # global 512-wide column strips
SW = N // NSTRIP     # 512 strip width
NLOC = 10            # local strips per core (window 2c..2c+9)
ND = 9               # diagonal offsets 0..8 per owned strip

TRACE = False
LAST_EXEC_NS = None
LAST_RESULTS = None

_nc_cache = None


def _build():
    global _nc_cache
    if _nc_cache is not None:
        return _nc_cache

    import concourse.tile as tile
    from concourse import bacc, mybir

    f32 = mybir.dt.float32
    f32r = mybir.dt.float32r
    AF = mybir.ActivationFunctionType
    Alu = mybir.AluOpType

    nc = bacc.Bacc("TRN2", target_bir_lowering=False)
    xj_d = nc.declare_dram_parameter("xj", [D, NLOC * SW], f32r, isOutput=False)
    on_d = nc.declare_dram_parameter("ones", [P, P], f32r, isOutput=False)
    # 18 row-groups (2 strips x 9 offsets) of [512, 512]
    out_d = nc.declare_dram_parameter("out", [2 * ND * SW, SW], f32, isOutput=True)

    with tile.TileContext(nc) as tc:
        with (
            tc.tile_pool(name="res", bufs=1) as res,
            tc.tile_pool(name="scr", bufs=1) as scr,
            tc.tile_pool(name="stg", bufs=4) as stg,
            tc.tile_pool(name="bnc", bufs=2) as bnc,
            tc.tile_pool(name="mmps", bufs=6, space="PSUM") as mmps,
            tc.tile_pool(name="auxps", bufs=2, space="PSUM") as auxps,
            tc.tile_pool(name="dscr", bufs=1, space="DRAM") as dpool,
        ):
            ones = res.tile([P, P], f32r, tag="ones")
            sqi_b = res.tile([P, 2 * SW], f32, tag="sqib")   # -0.5*||xi||^2, strips 0,1
            xj_sb = [
                res.tile([P, KO, SW], f32r, tag=f"xj{v}", name=f"xj{v}")
                for v in range(NLOC)
            ]
            sqj_t = [
                res.tile([P, KO], f32, tag=f"sqj{v}", name=f"sqj{v}")
                for v in range(NLOC)
            ]
            sq_dram = dpool.tile([1, NLOC * SW], f32, tag="sqrow")

            # ---- input DMAs: local strips in order (strips 0,1 first - the
            # moving operand and the norms everything needs) ----
            nc.sync.dma_start(ones, on_d[:])
            xj_ap = xj_d[:]
            for v in range(NLOC):
                nc.sync.dma_start(
                    xj_sb[v],
                    xj_ap[:, v * SW:(v + 1) * SW].rearrange(
                        "(ko p) j -> p ko j", p=P
                    ),
                )

            # ---- norms + main groups, interleaved by row strip so every
            # engine queue's order matches data arrival (strict-FIFO queues:
            # anything gated on a late strip must not precede work for an
            # early strip) ----
            out_v = out_d[:].rearrange("(g q p) i -> g p q i", q=KO, p=P)

            def norms(v):
                xsq = scr.tile([P, KO, SW], f32r, tag="xsq", name=f"xsq{v}")
                nc.scalar.activation(xsq, xj_sb[v].bitcast(f32), AF.Square)
                ps = auxps.tile([1, SW], f32, tag="aux", name=f"auxr{v}")
                for ko in range(KO):
                    nc.tensor.matmul(
                        ps, ones[:, 0:1], xsq[:, ko],
                        start=(ko == 0), stop=(ko == KO - 1),
                    )
                row = bnc.tile([1, SW], f32, tag="row", name=f"row{v}")
                nc.vector.tensor_copy(row, ps)
                nc.gpsimd.dma_start(sq_dram[:, v * SW:(v + 1) * SW], row)
                with nc.allow_non_contiguous_dma(reason="norms gather, 2KB"):
                    nc.gpsimd.dma_start(
                        sqj_t[v],
                        sq_dram[0, v * SW:(v + 1) * SW].rearrange(
                            "(t p) -> p t", p=P
                        ),
                    )
                if v < 2:
                    # -0.5*||xi||^2 broadcast for the moving strips
                    psb = auxps.tile([P, SW], f32, tag="aux", name=f"auxb{v}")
                    for ko in range(KO):
                        nc.tensor.matmul(
                            psb, ones, xsq[:, ko],
                            start=(ko == 0), stop=(ko == KO - 1),
                        )
                    nc.vector.tensor_scalar_mul(
                        sqi_b[:, v * SW:(v + 1) * SW], psb, -0.5
                    )

            def group(s, dd):
                rl = s + dd           # local index of the row strip
                stage = stg.tile([P, KO, SW], f32, tag="stage")
                for q in range(KO):
                    ps = mmps.tile(
                        [P, SW], f32, tag="mm", name=f"mm{s}_{dd}_{q}"
                    )
                    for ko in range(KO):
                        nc.tensor.matmul(
                            ps,
                            xj_sb[rl][:, ko, q * P:(q + 1) * P],
                            xj_sb[s][:, ko],
                            start=(ko == 0), stop=(ko == KO - 1),
                        )
                    nc.vector.tensor_tensor(
                        ps, ps, sqi_b[:, s * SW:(s + 1) * SW], Alu.add
                    )
                    nc.scalar.activation(
                        stage[:, q], ps,
                        AF.Sqrt, bias=sqj_t[rl][:, q:q + 1], scale=-2.0,
                    )
                nc.gpsimd.dma_start(out_v[s * ND + dd], stage)

            for rl in range(NLOC):
                norms(rl)
                if rl <= ND - 1:
                    group(0, rl)
                if rl >= 1:
                    group(1, rl - 1)

    nc.compile()
    _nc_cache = nc
    return nc


def kernel(embeddings):
    global LAST_EXEC_NS, LAST_RESULTS
    emb = np.ascontiguousarray(np.asarray(embeddings, dtype=np.float32))
    assert emb.shape == (N, D)
    xt = np.ascontiguousarray(emb.T)
    ones = np.ones((P, P), dtype=np.float32)
    in_maps = []
    for c in range(NCORES):
        strips = [(2 * c + k) % NSTRIP for k in range(NLOC)]
        xj = np.ascontiguousarray(
            np.concatenate([xt[:, s * SW:(s + 1) * SW] for s in strips], axis=1)
        )
        in_maps.append({"xj": xj, "ones": ones})

    nc = _build()
    from concourse.bass_utils import run_bass_kernel_spmd

    kwargs = {}
    if TRACE:
        kwargs["trace"] = True
    try:
        r = run_bass_kernel_spmd(
            nc, in_maps, core_ids=list(range(NCORES)), **kwargs
        )
    except Exception:  # noqa: BLE001
        # A previously-profiled NEFF can leave one-shot NRT state that fails
        # the next execution; the failed attempt clears it.
        r = run_bass_kernel_spmd(
            nc, in_maps, core_ids=list(range(NCORES)), **kwargs
        )
    LAST_EXEC_NS = r.exec_time_ns
    LAST_RESULTS = r

    full = np.empty((N, N), dtype=np.float32)
    for c in range(NCORES):
        arr = r.results[c]["out"]  # [18*512, 512]
        for s in range(2):
            sg = (2 * c + s) % NSTRIP          # global column strip
            for dd in range(ND):
                rg = (sg + dd) % NSTRIP        # global row strip
                blk = arr[(s * ND + dd) * SW:(s * ND + dd + 1) * SW, :]
                full[rg * SW:(rg + 1) * SW, sg * SW:(sg + 1) * SW] = blk
                full[sg * SW:(sg + 1) * SW, rg * SW:(rg + 1) * SW] = blk.T
    np.fill_diagonal(full, 0.0)
    return full[None, :, :]


# revision 12
# speedup vs baseline: 1.1982x; 1.1982x over previous
"""Symmetric-halved Euclidean distance matrix on 8 Trainium2 NeuronCores.

Decomposition: 16 column strips of 512. Core c owns strips 2c, 2c+1 and
computes, for each owned strip s, the blocks d(rows strip (s+d) mod 16,
cols strip s) for diagonal offsets d = 0..8. Every unordered strip pair
{u, v} is covered (offset (v-u) mod 16 <= 8 exactly once, except offset-8
pairs computed twice - harmless). The host mirrors each [512, 512] block to
its transposed position, so only ~59% of the matrix is computed on device.

The core's input is one local window xj = X^T columns for strips
2c..2c+9 (mod 16) [512, 5120]; all addressing inside the kernel uses local
strip indices 0..9, so the program is SPMD-uniform.
"""
import sys

sys.path.insert(0, "/opt/trn_rl_repo")

import numpy as np

N, D, NCORES = 8192, 512, 8
P = 128
KO = D // P          # 4 contraction blocks
NSTRIP = 16          # global 512-wide column strips
SW = N // NSTRIP     # 512 strip width
NLOC = 10            # local strips per core (window 2c..2c+9)
ND = 9               # diagonal offsets 0..8 per owned strip

TRACE = False
LAST_EXEC_NS = None
LAST_RESULTS = None

_nc_cache = None


def _build():
    global _nc_cache
    if _nc_cache is not None:
        return _nc_cache

    import concourse.tile as tile
    from concourse import bacc, mybir

    f32 = mybir.dt.float32
    f32r = mybir.dt.float32r
    AF = mybir.ActivationFunctionType
    Alu = mybir.AluOpType

    nc = bacc.Bacc("TRN2", target_bir_lowering=False)
    xj_d = nc.declare_dram_parameter("xj", [D, NLOC * SW], f32r, isOutput=False)
    on_d = nc.declare_dram_parameter("ones", [P, P], f32r, isOutput=False)
    # 18 row-groups (2 strips x 9 offsets) of [512, 512]
    out_d = nc.declare_dram_parameter("out", [2 * ND * SW, SW], f32, isOutput=True)

    with tile.TileContext(nc) as tc:
        with (
            tc.tile_pool(name="res", bufs=1) as res,
            tc.tile_pool(name="scr", bufs=1) as scr,
            tc.tile_pool(name="stg", bufs=4) as stg,
            tc.tile_pool(name="bnc", bufs=2) as bnc,
            tc.tile_pool(name="mmps", bufs=6, space="PSUM") as mmps,
            tc.tile_pool(name="auxps", bufs=2, space="PSUM") as auxps,
            tc.tile_pool(name="dscr", bufs=1, space="DRAM") as dpool,
        ):
            ones = res.tile([P, P], f32r, tag="ones")
            sqi_b = res.tile([P, 2 * SW], f32, tag="sqib")   # -0.5*||xi||^2, strips 0,1
            xj_sb = [
                res.tile([P, KO, SW], f32r, tag=f"xj{v}", name=f"xj{v}")
                for v in range(NLOC)
            ]
            sqj_t = [
                res.tile([P, KO], f32, tag=f"sqj{v}", name=f"sqj{v}")
                for v in range(NLOC)
            ]
            sq_dram = dpool.tile([1, NLOC * SW], f32, tag="sqrow")

            # ---- input DMAs: local strips in order (strips 0,1 first - the
            # moving operand and the norms everything needs) ----
            nc.sync.dma_start(ones, on_d[:])
            xj_ap = xj_d[:]
            for v in range(NLOC):
                nc.sync.dma_start(
                    xj_sb[v],
                    xj_ap[:, v * SW:(v + 1) * SW].rearrange(
                        "(ko p) j -> p ko j", p=P
                    ),
                )

            # ---- norms + main groups, interleaved by row strip so every
            # engine queue's order matches data arrival (strict-FIFO queues:
            # anything gated on a late strip must not precede work for an
            # early strip) ----
            out_v = out_d[:].rearrange("(g q p) i -> g p q i", q=KO, p=P)

            def norms(v):
                xsq = scr.tile([P, KO, SW], f32r, tag="xsq", name=f"xsq{v}")
                nc.scalar.activation(xsq, xj_sb[v].bitcast(f32), AF.Square)
                ps = auxps.tile([1, SW], f32, tag="aux", name=f"auxr{v}")
                for ko in range(KO):
                    nc.tensor.matmul(
                        ps, ones[:, 0:1], xsq[:, ko],
                        start=(ko == 0), stop=(ko == KO - 1),
                    )
                row = bnc.tile([1, SW], f32, tag="row", name=f"row{v}")
                nc.vector.tensor_copy(row, ps)
                nc.gpsimd.dma_start(sq_dram[:, v * SW:(v + 1) * SW], row)
                with nc.allow_non_contiguous_dma(reason="norms gather, 2KB"):
                    nc.gpsimd.dma_start(
                        sqj_t[v],
                        sq_dram[0, v * SW:(v + 1) * SW].rearrange(
                            "(t p) -> p t", p=P
                        ),
                    )
                if v < 2:
                    # -0.5*||xi||^2 broadcast for the moving strips
                    psb = auxps.tile([P, SW], f32, tag="aux", name=f"auxb{v}")
                    for ko in range(KO):
                        nc.tensor.matmul(
                            psb, ones, xsq[:, ko],
                            start=(ko == 0), stop=(ko == KO - 1),
                        )
                    nc.vector.tensor_scalar_mul(
                        sqi_b[:, v * SW:(v + 1) * SW], psb, -0.5
                    )

            def group(s, dd):
                rl = s + dd           # local index of the row strip
                stage = stg.tile([P, KO, SW], f32, tag="stage")
                for q in range(KO):
                    ps = mmps.tile(
                        [P, SW], f32, tag="mm", name=f"mm{s}_{dd}_{q}"
                    )
                    for ko in range(KO):
                        nc.tensor.matmul(
                            ps,
                            xj_sb[rl][:, ko, q * P:(q + 1) * P],
                            xj_sb[s][:, ko],
                            start=(ko == 0), stop=(ko == KO - 1),
                        )
                    nc.vector.tensor_tensor(
                        ps, ps, sqi_b[:, s * SW:(s + 1) * SW], Alu.add
                    )
                    nc.scalar.activation(
                        stage[:, q], ps,
                        AF.Sqrt, bias=sqj_t[rl][:, q:q + 1], scale=-2.0,
                    )
                nc.gpsimd.dma_start(out_v[s * ND + dd], stage)

            norms(0)
            norms(1)
            for rl in range(NLOC):
                if rl + 2 < NLOC:
                    norms(rl + 2)
                if rl <= ND - 1:
                    group(0, rl)
                if rl >= 1:
                    group(1, rl - 1)

    nc.compile()
    _nc_cache = nc
    return nc


def kernel(embeddings):
    global LAST_EXEC_NS, LAST_RESULTS
    emb = np.ascontiguousarray(np.asarray(embeddings, dtype=np.float32))
    assert emb.shape == (N, D)
    xt = np.ascontiguousarray(emb.T)
    ones = np.ones((P, P), dtype=np.float32)
    in_maps = []
    for c in range(NCORES):
        strips = [(2 * c + k) % NSTRIP for k in range(NLOC)]
        xj = np.ascontiguousarray(
            np.concatenate([xt[:, s * SW:(s + 1) * SW] for s in strips], axis=1)
        )
        in_maps.append({"xj": xj, "ones": ones})

    nc = _build()
    from concourse.bass_utils import run_bass_kernel_spmd

    kwargs = {}
    if TRACE:
        kwargs["trace"] = True
    try:
        r = run_bass_kernel_spmd(
            nc, in_maps, core_ids=list(range(NCORES)), **kwargs
        )
    except Exception:  # noqa: BLE001
        # A previously-profiled NEFF can leave one-shot NRT state that fails
        # the next execution; the failed attempt clears it.
        r = run_bass_kernel_spmd(
            nc, in_maps, core_ids=list(range(NCORES)), **kwargs
        )
    LAST_EXEC_NS = r.exec_time_ns
    LAST_RESULTS = r

    full = np.empty((N, N), dtype=np.float32)
    for c in range(NCORES):
        arr = r.results[c]["out"]  # [18*512, 512]
        for s in range(2):
            sg = (2 * c + s) % NSTRIP          # global column strip
            for dd in range(ND):
                rg = (sg + dd) % NSTRIP        # global row strip
                blk = arr[(s * ND + dd) * SW:(s * ND + dd + 1) * SW, :]
                full[rg * SW:(rg + 1) * SW, sg * SW:(sg + 1) * SW] = blk
                full[sg * SW:(sg + 1) * SW, rg * SW:(rg + 1) * SW] = blk.T
    np.fill_diagonal(full, 0.0)
    return full[None, :, :]
